# revision 28
# baseline (speedup 1.0000x reference)
"""Couplformer attention kernel for 8 Trainium2 NeuronCores (Bass/Tile).

Shapes: x [16, 4096, 384], W_qkv [1152, 384], b_qkv [1152],
W_proj [384, 384], b_proj [384].  Data-parallel: B=16 -> 2 batches/core.

Per-core pipeline (bf16 compute, fp32 PSUM):
  1. xT via DRAM->SBUF DMA-transpose.
  2. token-major qkv projection (lhsT = xT chunks, rhs = W_qkv^T).
  3. scatter-write qkv to DRAM: l1d [y, w, (h,c)] (2.3KB runs).
  4. per 4-head group: staged reads of L1q|L1k, L4q|L4k (from l1d with a
     transposed-nest AP), L1v.
  5. per head: DVE/ACT slice-copies -> packed [128, 4096] tile -> ONE
     DMA-transpose -> 32 contraction chunks [128, 64|64] giving both
     height-logits (L2 = [(w,c), y]) and width-logits (L3 = [(y,c), w]).
  6. chunked logits matmuls (16 x K=128 N=64, PSUM-accumulated), softmax
     via ACT exp (scale=HD^-0.25) with fused row-sum, PE-transpose of A/B.
  7. MM#1: T2[u,(j,c)] = A^T-weighted sum of V (strided rhs, N=512).
  8. group fold W1 -> DRAM -> W1r [j, (u,h,c)]; MM#2 with B^T.
  9. fold attention output -> channel-major xcm_d in DRAM (DRAM->DRAM).
 10. load xcm, token-major output projection, scattered token writes.

The two batches are software-pipelined: proj(b1) tiles are emitted between
batch-0 attention groups, and outproj(b0) tiles between batch-1 groups, so
the TensorE always has dense matmul work to fill attention stalls.

Biases are applied exactly via K=1 ones-row matmuls.
"""

import numpy as np

B, N, C = 16, 4096, 384
NH, HD = 12, 32
HT, WD = 64, 64
SCALE = float(HD ** (-0.25))
NCORES = 8
BL = B // NCORES  # 2 batches per core
HG = 4            # heads per group
NG = NH // HG     # 3 groups

_CACHE = {}


def _build_nc(with_bias=True):
    import concourse.bacc as bacc
    import concourse.mybir as mybir
    from concourse import tile
    from concourse.masks import make_identity

    BF16 = mybir.dt.bfloat16
    F32 = mybir.dt.float32
    AF = mybir.ActivationFunctionType

    nc = bacc.Bacc("TRN2", target_bir_lowering=False, debug=False,
                   enable_asserts=False, num_devices=NCORES)

    x_d = nc.dram_tensor("x_d", [BL * C, N], BF16, kind="ExternalInput").ap()
    wq_d = nc.dram_tensor("wq_d", [C, 3 * C], BF16, kind="ExternalInput").ap()
    wp_d = nc.dram_tensor("wp_d", [C, C], BF16, kind="ExternalInput").ap()
    bq_d = nc.dram_tensor("bq_d", [1, 3 * C], BF16, kind="ExternalInput").ap()
    bp_d = nc.dram_tensor("bp_d", [1, C], BF16, kind="ExternalInput").ap()
    out_d = nc.dram_tensor("out_d", [BL * N, C], BF16, kind="ExternalOutput").ap()

    with tile.TileContext(nc) as tc:
        with (
            tc.tile_pool(name="const", bufs=1) as constp,
            tc.tile_pool(name="xtp", bufs=1) as xtp,
            tc.tile_pool(name="tmp", bufs=4) as tmp,
            tc.tile_pool(name="grp", bufs=1) as grp,
            tc.tile_pool(name="headp", bufs=2) as headp,
            tc.tile_pool(name="abp", bufs=4) as abp,
            tc.tile_pool(name="smallp", bufs=4) as smallp,
            tc.tile_pool(name="btp", bufs=8) as btp,
            tc.tile_pool(name="xcmp", bufs=1) as xcmp,
            tc.tile_pool(name="outp", bufs=4) as outp,
            tc.tile_pool(name="ps", bufs=8, space="PSUM") as ps,
            tc.tile_pool(name="dram", bufs=1, space="DRAM") as dram,
        ):
            # ---- constants
            wq = constp.tile([128, 3, 3 * C], BF16)   # [cin%128, cc, cout]
            nc.sync.dma_start(wq[:], wq_d.rearrange("(cc p) co -> p cc co", p=128))
            wp = constp.tile([128, 3, C], BF16)
            nc.sync.dma_start(wp[:], wp_d.rearrange("(cc p) co -> p cc co", p=128))
            bq = constp.tile([1, 3 * C], BF16)
            nc.sync.dma_start(bq[:], bq_d)
            bp = constp.tile([1, C], BF16)
            nc.sync.dma_start(bp[:], bp_d)
            ident = constp.tile([128, 128], BF16)
            make_identity(nc, ident[:])
            ones = constp.tile([1, 128], BF16)
            nc.gpsimd.memset(ones[:], 1.0)

            def emit_xt(b):
                xt = xtp.tile([128, 3, N], BF16, tag="xt", name="xt")
                nc.sync.dma_start(
                    xt[:],
                    x_d[b * C:(b + 1) * C, :].rearrange("(cc p) n -> p cc n", p=128),
                )
                return xt

            def emit_proj_tile(xt, l1d, t):
                tmt = tmp.tile([128, 3 * C], BF16, tag="tmt", name="tmt")
                for co in range(3):
                    acc = ps.tile([128, 384], F32, tag="pp", name="acc")
                    for cc in range(3):
                        nc.tensor.matmul(
                            acc[:],
                            xt[:, cc, t * 128:(t + 1) * 128],
                            wq[:, cc, co * 384:(co + 1) * 384],
                            start=(cc == 0),
                            stop=(cc == 2 and not with_bias),
                        )
                    if with_bias:
                        nc.tensor.matmul(
                            acc[:], ones[:, 0:128],
                            bq[:, co * 384:(co + 1) * 384],
                            start=False, stop=True,
                        )
                    if co < 2:
                        nc.vector.tensor_copy(tmt[:, co * 384:(co + 1) * 384], acc[:])
                    else:
                        nc.scalar.activation(tmt[:, co * 384:(co + 1) * 384], acc[:], AF.Copy)
                # rows of tile t are tok = t*128 + p: y = 2t + p//64, w = p%64
                nc.sync.dma_start(
                    l1d.rearrange("y w hc -> (y w) hc")[t * 128:(t + 1) * 128, :],
                    tmt[:],
                )

            def emit_group_load(l1d, g):
                # ---- group reads (q -> parts 0:64, k -> parts 64:128)
                t1 = grp.tile([128, WD, HG * HD], BF16, tag="t1", name="t1")
                nc.gpsimd.dma_start(t1[0:64], l1d[:, :, g * 128:(g + 1) * 128])
                nc.gpsimd.dma_start(t1[64:128], l1d[:, :, 384 + g * 128:384 + (g + 1) * 128])
                t2 = grp.tile([128, HT, HG * HD], BF16, tag="t2", name="t2")
                nc.gpsimd.dma_start(
                    t2[0:64],
                    l1d.rearrange("y w hc -> w y hc")[:, :, g * 128:(g + 1) * 128],
                )
                nc.gpsimd.dma_start(
                    t2[64:128],
                    l1d.rearrange("y w hc -> w y hc")[:, :, 384 + g * 128:384 + (g + 1) * 128],
                )
                tv = grp.tile([64, WD, HG * HD], BF16, tag="tv", name="tv")
                nc.gpsimd.dma_start(tv[:], l1d[:, :, 768 + g * 128:768 + (g + 1) * 128])

                w1all = grp.tile([64, WD, HG, HD], BF16, tag="wo", name="w1all")
                t1v = t1.rearrange("p w (hh c) -> p w hh c", hh=HG)
                t2v = t2.rearrange("p w (hh c) -> p w hh c", hh=HG)

                # pass A: slice-copies + DMA-T for all heads
                abs_ = []
                for hh in range(HG):
                    qk = headp.tile([128, 2, WD * HD], BF16, tag="qk", name="qk")
                    qkv_ = qk.rearrange("p a (w c) -> p a w c", c=HD)
                    nc.vector.tensor_copy(qkv_[0:64, 0], t1v[0:64, :, hh, :])
                    nc.vector.tensor_copy(qkv_[64:128, 0], t1v[64:128, :, hh, :])
                    nc.vector.tensor_copy(qkv_[0:64, 1], t2v[0:64, :, hh, :])
                    nc.vector.tensor_copy(qkv_[64:128, 1], t2v[64:128, :, hh, :])
                    ab = abp.tile([128, 32, 128], BF16, tag="ab", name="ab")
                    nc.sync.dma_start(ab[:], qk.rearrange("p a f -> p (a f)"),
                                      transpose=True)
                    abs_.append(ab)
                return t1, tv, w1all, abs_

            def emit_head_compute(tv, w1all, ab, hh):
                mats = []
                for s in range(2):
                    lg = ps.tile([64, 64], F32, tag="pp", name="lg")
                    for t in range(16):
                        nc.tensor.matmul(
                            lg[:],
                            ab[:, s * 16 + t, 0:64],
                            ab[:, s * 16 + t, 64:128],
                            start=(t == 0), stop=(t == 15),
                        )
                    exps = smallp.tile([64, 64], F32, tag="exps", name="exps")
                    ssum = smallp.tile([64, 1], F32, tag="ssum", name="ssum")
                    nc.scalar.activation(exps[:], lg[:], AF.Exp,
                                         scale=SCALE, accum_out=ssum[:])
                    rsum = smallp.tile([64, 1], F32, tag="rsum", name="rsum")
                    nc.vector.reciprocal(rsum[:], ssum[:])
                    amat = smallp.tile([64, 64], BF16, tag="amat", name="amat")
                    nc.vector.tensor_scalar_mul(amat[:], exps[:], rsum[:])
                    tps = ps.tile([64, 64], BF16, tag="pp", name="tps")
                    nc.tensor.transpose(tps[:], amat[:], ident[0:64, 0:64])
                    if s == 0:
                        tmat = smallp.tile([64, 64], BF16, tag="atb", name="atb")
                    else:
                        tmat = btp.tile([64, 64], BF16, tag="btb", name="btb")
                    nc.vector.tensor_copy(tmat[:], tps[:])
                    mats.append(tmat)
                atb, btb = mats

                tvv = tv.rearrange("p w (hh c) -> p w hh c", hh=HG)
                for q4 in range(4):
                    w1p = ps.tile([64, 512], F32, tag="pp", name="w1p")
                    nc.tensor.matmul(
                        w1p[:], atb[:],
                        tvv[:, q4 * 16:(q4 + 1) * 16, hh, :],
                        start=True, stop=True,
                    )
                    dst = w1all[:, q4 * 16:(q4 + 1) * 16, hh, :]
                    src = w1p.rearrange("p (j c) -> p j c", c=HD)
                    if q4 % 2 == 0:
                        nc.vector.tensor_copy(dst, src)
                    else:
                        nc.scalar.activation(dst, src, AF.Copy)
                return btb

            def emit_group_tail(xcm_d, g, w1all, bts, fill):
                # ---- group fold W1 -> DRAM -> W1r, then MM#2
                w1d = dram.tile([64, WD * HG * HD], BF16, tag="w1d", bufs=2, name="w1d")
                nc.gpsimd.dma_start(w1d[:], w1all.rearrange("p j hh c -> p (j hh c)"))
                w1r = grp.tile([64, 64, HG, HD], BF16, tag="w1r", name="w1r")
                nc.gpsimd.dma_start(
                    w1r.rearrange("j u hh c -> j u (hh c)"),
                    w1d.rearrange("u (j hc) -> j u hc", j=WD),
                )
                fill(2)
                o2all = grp.tile([64, HG, HD, 64], BF16, tag="wo", name="o2all")
                for hh in range(HG):
                    for q4 in range(4):
                        op = ps.tile([64, 512], F32, tag="pp", name="op")
                        nc.tensor.matmul(
                            op[:], bts[hh][:],
                            w1r[:, q4 * 16:(q4 + 1) * 16, hh, :],
                            start=True, stop=True,
                        )
                        dst = o2all[:, hh, :, q4 * 16:(q4 + 1) * 16]
                        src = op.rearrange("p (u c) -> p c u", c=HD)
                        if (hh + q4) % 2 == 0:
                            nc.vector.tensor_copy(dst, src)
                        else:
                            nc.scalar.activation(dst, src, AF.Copy)

                # ---- evacuate O2 to DRAM fast, then permute DRAM->DRAM
                o2d = dram.tile([64, HG * HD * 64], BF16, tag="o2d", bufs=2, name="o2d")
                nc.scalar.dma_start(o2d[:], o2all.rearrange("p hh c u -> p (hh c u)"))
                for hh in range(HG):
                    h = g * HG + hh
                    xeng = nc.sync if hh % 2 == 0 else nc.scalar
                    xeng.dma_start(
                        xcm_d[h * HD:(h + 1) * HD].rearrange("p (i u) -> p i u", i=64),
                        o2d.rearrange("i (hh c u) -> hh c i u", hh=HG, c=HD)[hh],
                    )

            def emit_load_xcm(xcm_d):
                xcm = [
                    xcmp.tile([128, N], BF16, tag=f"xcm{cc}", name=f"xcm{cc}")
                    for cc in range(3)
                ]
                for cc in range(3):
                    eng = nc.sync if cc % 2 == 0 else nc.scalar
                    eng.dma_start(xcm[cc][:], xcm_d[cc * 128:(cc + 1) * 128, :])
                return xcm

            def emit_outproj_tile(b, xcm, t):
                acc = ps.tile([128, C], F32, tag="pp", name="acc2")
                for cc in range(3):
                    nc.tensor.matmul(
                        acc[:],
                        xcm[cc][:, t * 128:(t + 1) * 128],
                        wp[:, cc, :],
                        start=(cc == 0),
                        stop=(cc == 2 and not with_bias),
                    )
                if with_bias:
                    nc.tensor.matmul(acc[:], ones[:, 0:128], bp[:],
                                     start=False, stop=True)
                ot = outp.tile([128, C], BF16, tag="ot", name="ot")
                if t % 2 == 0:
                    nc.vector.tensor_copy(ot[:], acc[:])
                else:
                    nc.scalar.activation(ot[:], acc[:], AF.Copy)
                # permuted layout: row = b*4096 + t*128 + p ; host unpermutes
                nc.scalar.dma_start(
                    out_d[b * N + t * 128:b * N + (t + 1) * 128, :], ot[:]
                )

            # ================= interleaved schedule =================
            l1ds, xcmds = [], []
            for b in range(BL):
                l1ds.append(dram.tile([HT, WD, 3 * C], BF16, tag="l1d", bufs=2,
                                      name="l1d"))
                xcmds.append(dram.tile([3 * 128, N], BF16, tag="xcmd", bufs=2,
                                       name="xcmd"))

            xt0 = emit_xt(0)
            for t in range(32):
                emit_proj_tile(xt0, l1ds[0], t)
            xt1 = emit_xt(1)

            class Filler:
                def __init__(self, fn, n):
                    self.fn, self.n, self.i = fn, n, 0
                def __call__(self, k):
                    while k > 0 and self.i < self.n:
                        self.fn(self.i)
                        self.i += 1
                        k -= 1
                def drain(self):
                    self(self.n)

            def attn_batch(bb, l1d, xcm_d, fill):
                for g in range(NG):
                    t1, tv, w1all, abs_ = emit_group_load(l1d, g)
                    fill(3)
                    bts = []
                    for hh in range(HG):
                        bts.append(emit_head_compute(tv, w1all, abs_[hh], hh))
                        fill(1)
                    emit_group_tail(xcm_d, g, w1all, bts, fill)
                    fill(1)

            fill1 = Filler(lambda t: emit_proj_tile(xt1, l1ds[1], t), 32)
            attn_batch(0, l1ds[0], xcmds[0], fill1)
            fill1.drain()

            xcm0 = emit_load_xcm(xcmds[0])
            fill2 = Filler(lambda t: emit_outproj_tile(0, xcm0, t), 32)
            attn_batch(1, l1ds[1], xcmds[1], fill2)
            fill2.drain()

            xcm1 = emit_load_xcm(xcmds[1])
            for t in range(32):
                emit_outproj_tile(1, xcm1, t)

    nc.compile()
    return nc


def _get_nc(with_bias=True):
    key = f"nc{int(with_bias)}"
    if key not in _CACHE:
        _CACHE[key] = _build_nc(with_bias)
    return _CACHE[key]


def _run_on_hw(x, W_qkv, b_qkv, W_proj, b_proj, trace=False):
    import ml_dtypes
    from concourse.bass_utils import run_bass_kernel_spmd

    bf16 = ml_dtypes.bfloat16
    with_bias = bool(np.any(b_qkv) or np.any(b_proj))
    nc = _get_nc(with_bias)
    wq = np.ascontiguousarray(W_qkv.T).astype(bf16)      # [384, 1152]
    wp = np.ascontiguousarray(W_proj.T).astype(bf16)     # [384, 384]
    bqv = b_qkv.reshape(1, -1).astype(bf16)
    bpv = b_proj.reshape(1, -1).astype(bf16)
    xs = np.ascontiguousarray(
        x.reshape(NCORES, BL, N, C).transpose(0, 1, 3, 2)
    ).reshape(NCORES, BL * C, N).astype(bf16)
    in_maps = [
        {"x_d": xs[c], "wq_d": wq, "bq_d": bqv, "wp_d": wp, "bp_d": bpv}
        for c in range(NCORES)
    ]
    res = run_bass_kernel_spmd(nc, in_maps, core_ids=list(range(NCORES)),
                               trace=trace)
    out = np.stack([res.results[c]["out_d"] for c in range(NCORES)])
    # rows are (b, t, i', u); true token = u*64 + (2t + i')
    out = out.reshape(NCORES, BL, 64, 64, C).transpose(0, 1, 3, 2, 4)
    out = out.astype(np.float32).reshape(B, N, C)
    return out, res


def kernel(x, W_qkv, b_qkv, W_proj, b_proj):
    x = np.asarray(x, dtype=np.float32)
    W_qkv = np.asarray(W_qkv, dtype=np.float32)
    b_qkv = np.asarray(b_qkv, dtype=np.float32)
    W_proj = np.asarray(W_proj, dtype=np.float32)
    b_proj = np.asarray(b_proj, dtype=np.float32)
    out, _ = _run_on_hw(x, W_qkv, b_qkv, W_proj, b_proj, trace=False)
    return out
